# revision 3
# baseline (speedup 1.0000x reference)
"""Trainium2 Bass kernel for nn_DeepCluster (vq_codebook).

Computation (per row x of shape [72]):
  7-layer MLP (Linear chain, ReLU after layers 2 and 4) -> f [200]
  sq[j]  = |f|^2 - 2*(f @ center)[j] + |center[:, j]|^2      (center: [200, 72])
  nom    = 1 / (1 + sq)                                       (alpha = 1)
  q      = nom / sum_j nom

Strategy: pure data parallel over 8 NeuronCores (batch split).  On each
core, activations flow as [features(partitions), batch(free)] bf16 tiles
of 512 rows; bf16 matmuls stream at 1 cycle/row on the PE with fast
weight load.  The distance computation runs transposed ([cluster,
batch]) so its matmuls also get a 512-wide moving operand; |c_j|^2 + 1
is added per-partition in f32 on the DVE (it dominates sq, so it must
not be rounded to bf16), and the value path after the reciprocal stays
f32.  PSUM->SBUF epilogues (bias add + optional ReLU) are split between
the Scalar and Vector engines to keep both under the PE's critical path.
"""

import numpy as np

DIMS = [72, 128, 256, 256, 512, 512, 512, 200]
RELU_LAYERS = {2, 4}  # 1-indexed layers followed by ReLU
N_CORES = 8
N_FULL = 262144
B = 512  # rows per pipeline tile
P = 128

_CACHE = {}


def _build(n_rows):
    import concourse.mybir as mybir
    from concourse import bacc
    from concourse.tile import TileContext
    from concourse.masks import make_identity

    f32 = mybir.dt.float32
    bf16 = mybir.dt.bfloat16
    AF = mybir.ActivationFunctionType
    AX = mybir.AxisListType
    ALU = mybir.AluOpType

    kc_l = [(DIMS[i] + 127) // 128 for i in range(7)]
    mc_l = [(DIMS[i + 1] + 127) // 128 for i in range(7)]

    nc = bacc.Bacc(None, target_bir_lowering=False, debug=False)
    x_d = nc.dram_tensor("x", [n_rows, 72], f32, kind="ExternalInput")
    q_d = nc.dram_tensor("q", [n_rows, 72], f32, kind="ExternalOutput")
    w_d, b_d = [], []
    for l in range(7):
        din, dout = DIMS[l], DIMS[l + 1]
        w_d.append(
            nc.dram_tensor(
                f"w{l + 1}", [min(din, 128), kc_l[l] * dout], bf16, kind="ExternalInput"
            )
        )
        b_d.append(nc.dram_tensor(f"b{l + 1}", [128, mc_l[l]], f32, kind="ExternalInput"))
    cm2A_d = nc.dram_tensor("cm2A", [128, 72], bf16, kind="ExternalInput")
    cm2B_d = nc.dram_tensor("cm2B", [72, 72], bf16, kind="ExternalInput")
    csq1_d = nc.dram_tensor("csq1", [72, 1], f32, kind="ExternalInput")

    n_tiles = n_rows // B
    assert n_rows % B == 0

    with TileContext(nc) as tc:
        with (
            tc.tile_pool(name="consts", bufs=1) as consts,
            tc.tile_pool(name="acts", bufs=2) as acts,
            tc.tile_pool(name="pmm", bufs=3, space="PSUM") as pmm,
            tc.tile_pool(name="ptp", bufs=2, space="PSUM") as ptp,
            tc.tile_pool(name="ptail", bufs=3, space="PSUM") as ptail,
        ):
            ones = consts.tile([128, 72], bf16, tag="ones")
            nc.vector.memset(ones, 1.0)
            ident = consts.tile([128, 128], f32, tag="ident")
            make_identity(nc, ident)
            cm2A = consts.tile([128, 72], bf16, tag="cm2A")
            nc.sync.dma_start(out=cm2A, in_=cm2A_d[:])
            cm2B = consts.tile([72, 72], bf16, tag="cm2B")
            nc.sync.dma_start(out=cm2B, in_=cm2B_d[:])
            csq1 = consts.tile([72, 1], f32, tag="csq1")
            nc.sync.dma_start(out=csq1, in_=csq1_d[:])
            w_sb, b_sb = [], []
            for l in range(7):
                wt = consts.tile(list(w_d[l].shape), bf16, tag=f"w{l}")
                nc.sync.dma_start(out=wt, in_=w_d[l][:])
                w_sb.append(wt)
                bt = consts.tile([128, mc_l[l]], f32, tag=f"bias{l}")
                nc.sync.dma_start(out=bt, in_=b_d[l][:])
                b_sb.append(bt)

            x_r = x_d[:].rearrange("(t c p) j -> t p c j", p=P, c=B // P)

            for t in range(n_tiles):
                # ---- load + transpose input tile: [512, 72] -> [72, 512]
                x_sb = acts.tile([P, B // P, 72], f32, tag="x")
                nc.sync.dma_start(out=x_sb, in_=x_r[t])
                xT = acts.tile([72, B], bf16, tag="xT")
                for c in range(B // P):
                    pt = ptp.tile([72, P], f32, tag="xtp")
                    nc.tensor.transpose(pt, x_sb[:, c, :], ident)
                    nc.vector.tensor_copy(xT[:, P * c : P * (c + 1)], pt)

                # ---- MLP chain, activations as [D(part), B(free)] bf16
                h = [xT]
                ep = 0  # epilogue chunk counter for ACT/DVE balancing
                for l in range(7):
                    dout = DIMS[l + 1]
                    kc, mc = kc_l[l], mc_l[l]
                    relu = (l + 1) in RELU_LAYERS
                    hn = []
                    for m in range(mc):
                        pw = min(128, dout - 128 * m)
                        ps = pmm.tile([pw, B], f32, tag="mm")
                        for k in range(kc):
                            lhsT = w_sb[l][:, k * dout + 128 * m : k * dout + 128 * m + pw]
                            nc.tensor.matmul(
                                ps, lhsT, h[k], start=(k == 0), stop=(k == kc - 1)
                            )
                        ht = acts.tile([pw, B], bf16, tag=f"h{l + 1}m{m}")
                        bias_col = b_sb[l][:pw, m : m + 1]
                        if ep % 3 != 2:  # scalar engine (ACT)
                            nc.scalar.activation(
                                out=ht,
                                in_=ps,
                                func=AF.Relu if relu else AF.Identity,
                                bias=bias_col,
                                scale=1.0,
                            )
                        else:  # vector engine (DVE)
                            if relu:
                                nc.vector.tensor_scalar(
                                    out=ht,
                                    in0=ps,
                                    scalar1=bias_col,
                                    scalar2=0.0,
                                    op0=ALU.add,
                                    op1=ALU.max,
                                )
                            else:
                                nc.vector.tensor_scalar_add(ht, ps, bias_col)
                        ep += 1
                        hn.append(ht)
                    h = hn

                f0, f1 = h  # [128, B], [72, B] bf16
                g0 = acts.tile([128, B], bf16, tag="g0")
                nc.vector.tensor_mul(g0, f0, f0)
                g1 = acts.tile([72, B], bf16, tag="g1")
                nc.vector.tensor_mul(g1, f1, f1)

                # ---- sdT[j, n] = |f_n|^2 - 2 cross (PSUM, f32)
                sdT = ptail.tile([72, B], f32, tag="tail")
                nc.tensor.matmul(sdT, ones[:128, :72], g0, start=True, stop=False)
                nc.tensor.matmul(sdT, ones[:72, :72], g1, start=False, stop=False)
                nc.tensor.matmul(sdT, cm2A, f0, start=False, stop=False)
                nc.tensor.matmul(sdT, cm2B, f1, start=False, stop=True)

                # ---- += (1 + |c_j|^2) per partition (f32, exact), reciprocal
                sd1 = acts.tile([72, B], f32, tag="sd1")
                nc.vector.tensor_scalar_add(sd1, sdT, csq1[:, 0:1])
                nomT = acts.tile([72, B], f32, tag="nomT")
                nc.vector.reciprocal(nomT, sd1)

                # ---- transpose back per 128-row chunk, normalize, store
                for s in range(B // P):
                    pq = ptail.tile([P, 72], f32, tag="tail")
                    nc.tensor.transpose(pq, nomT[:, P * s : P * (s + 1)], ident[:72, :72])
                    rs = acts.tile([P, 1], f32, tag=f"rs{s}")
                    nc.vector.reduce_sum(rs, pq, axis=AX.X)
                    rr = acts.tile([P, 1], f32, tag=f"rr{s}")
                    nc.vector.reciprocal(rr, rs)
                    qt = acts.tile([P, 72], f32, tag=f"qt{s}")
                    nc.scalar.activation(out=qt, in_=pq, func=AF.Copy, scale=rr)
                    nc.sync.dma_start(
                        out=q_d[B * t + P * s : B * t + P * (s + 1), :], in_=qt
                    )

    nc.compile()
    return nc


def _prep_consts(ws, bs, center):
    """Host-side marshalling of the small replicated weights."""
    import ml_dtypes

    bf = ml_dtypes.bfloat16
    kc_l = [(DIMS[i] + 127) // 128 for i in range(7)]
    mc_l = [(DIMS[i + 1] + 127) // 128 for i in range(7)]
    consts = {}
    for l in range(7):
        din, dout = DIMS[l], DIMS[l + 1]
        w = np.ascontiguousarray(ws[l], dtype=np.float32)
        if din > 128:
            kc = kc_l[l]
            w = np.ascontiguousarray(
                w.reshape(kc, 128, dout).transpose(1, 0, 2).reshape(128, kc * dout)
            )
        consts[f"w{l + 1}"] = w.astype(bf)
        bt = np.zeros((128, mc_l[l]), dtype=np.float32)
        for m in range(mc_l[l]):
            pw = min(128, dout - 128 * m)
            bt[:pw, m] = bs[l][128 * m : 128 * m + pw]
        consts[f"b{l + 1}"] = bt
    c = np.asarray(center, dtype=np.float32)
    consts["cm2A"] = np.ascontiguousarray(-2.0 * c[:128, :]).astype(bf)
    consts["cm2B"] = np.ascontiguousarray(-2.0 * c[128:, :]).astype(bf)
    consts["csq1"] = np.ascontiguousarray(
        (1.0 + (c.astype(np.float64) ** 2).sum(axis=0)).reshape(72, 1)
    ).astype(np.float32)
    return consts


def kernel(
    inputs, w1, b1, w2, b2, w3, b3, w4, b4, w5, b5, w6, b6, w7, b7, center
):
    from concourse.bass_utils import run_bass_kernel_spmd

    x = np.ascontiguousarray(np.asarray(inputs), dtype=np.float32)
    n = x.shape[0]
    n_loc = n // N_CORES
    key = n_loc
    if key not in _CACHE:
        _CACHE[key] = _build(n_loc)
    nc = _CACHE[key]

    consts = _prep_consts(
        [w1, w2, w3, w4, w5, w6, w7], [b1, b2, b3, b4, b5, b6, b7], center
    )
    in_maps = []
    for c in range(N_CORES):
        m = {"x": np.ascontiguousarray(x[c * n_loc : (c + 1) * n_loc])}
        m.update(consts)
        in_maps.append(m)
    res = run_bass_kernel_spmd(nc, in_maps, core_ids=list(range(N_CORES)))
    return np.concatenate([res.results[c]["q"] for c in range(N_CORES)], axis=0)


# revision 5
# speedup vs baseline: 1.2376x; 1.2376x over previous
"""Trainium2 Bass kernel for nn_DeepCluster (vq_codebook).

Computation (per row x of shape [72]):
  7-layer MLP (Linear chain, ReLU after layers 2 and 4) -> f [200]
  sq[j]  = |f|^2 - 2*(f @ center)[j] + |center[:, j]|^2      (center: [200, 72])
  nom    = 1 / (1 + sq)                                       (alpha = 1)
  q      = nom / sum_j nom

Strategy: pure data parallel over 8 NeuronCores (batch split).  On each
core, activations flow as [features(partitions), batch(free)] bf16 tiles
of 512 rows; bf16 matmuls stream at 1 cycle/row on the PE with fast
weight load.  The distance computation runs transposed ([cluster,
batch]) so its matmuls also get a 512-wide moving operand; |c_j|^2 + 1
is added per-partition in f32 (it dominates sq, so it must not be
rounded to bf16), and the value path after the reciprocal stays f32.
PSUM->SBUF epilogues (bias add + optional ReLU) are split between the
Scalar and Vector engines.  The per-tile tail (reciprocal -> transpose
back -> row-normalize -> store) is software-pipelined one tile behind
the matmul stage so the PE never waits on the DVE round trip.
"""

import numpy as np

DIMS = [72, 128, 256, 256, 512, 512, 512, 200]
RELU_LAYERS = {2, 4}  # 1-indexed layers followed by ReLU
N_CORES = 8
N_FULL = 262144
B = 512  # rows per pipeline tile
P = 128

_CACHE = {}


def _build(n_rows):
    import concourse.bass as bass
    import concourse.mybir as mybir
    from concourse import bacc
    from concourse.tile import TileContext
    from concourse.masks import make_identity

    f32 = mybir.dt.float32
    bf16 = mybir.dt.bfloat16
    AF = mybir.ActivationFunctionType
    AX = mybir.AxisListType
    ALU = mybir.AluOpType

    kc_l = [(DIMS[i] + 127) // 128 for i in range(7)]
    mc_l = [(DIMS[i + 1] + 127) // 128 for i in range(7)]

    nc = bacc.Bacc(None, target_bir_lowering=False, debug=False)
    x_d = nc.dram_tensor("x", [n_rows, 72], bf16, kind="ExternalInput")
    q_d = nc.dram_tensor("q", [n_rows, 72], f32, kind="ExternalOutput")
    w_d, b_d = [], []
    for l in range(7):
        din, dout = DIMS[l], DIMS[l + 1]
        w_d.append(
            nc.dram_tensor(
                f"w{l + 1}", [min(din, 128), kc_l[l] * dout], bf16, kind="ExternalInput"
            )
        )
        b_d.append(nc.dram_tensor(f"b{l + 1}", [128, mc_l[l]], f32, kind="ExternalInput"))
    cm2A_d = nc.dram_tensor("cm2A", [128, 72], bf16, kind="ExternalInput")
    cm2B_d = nc.dram_tensor("cm2B", [72, 72], bf16, kind="ExternalInput")
    csq1_d = nc.dram_tensor("csq1", [72, 1], f32, kind="ExternalInput")

    n_tiles = n_rows // B
    assert n_rows % B == 0
    C = B // P  # 128-row chunks per tile

    with TileContext(nc) as tc:
        with (
            tc.tile_pool(name="consts", bufs=1) as consts,
            tc.tile_pool(name="acts", bufs=2) as acts,
            tc.tile_pool(name="pmm", bufs=3, space="PSUM") as pmm,
            tc.tile_pool(name="ptp", bufs=2, space="PSUM") as ptp,
            tc.tile_pool(name="psd", bufs=2, space="PSUM") as psd,
            tc.tile_pool(name="ppq", bufs=1, space="PSUM") as ppq,
        ):
            ones = consts.tile([128, 72], bf16, tag="ones")
            nc.vector.memset(ones, 1.0)
            ident = consts.tile([128, 128], bf16, tag="ident")
            make_identity(nc, ident)
            identf = consts.tile([128, 128], f32, tag="identf")
            make_identity(nc, identf)
            cm2A = consts.tile([128, 72], bf16, tag="cm2A")
            nc.sync.dma_start(out=cm2A, in_=cm2A_d[:])
            cm2B = consts.tile([72, 72], bf16, tag="cm2B")
            nc.sync.dma_start(out=cm2B, in_=cm2B_d[:])
            csq1 = consts.tile([72, 1], f32, tag="csq1")
            nc.sync.dma_start(out=csq1, in_=csq1_d[:])
            w_sb, b_sb = [], []
            for l in range(7):
                wt = consts.tile(list(w_d[l].shape), bf16, tag=f"w{l}")
                nc.sync.dma_start(out=wt, in_=w_d[l][:])
                w_sb.append(wt)
                bt = consts.tile([128, mc_l[l]], f32, tag=f"bias{l}")
                nc.sync.dma_start(out=bt, in_=b_d[l][:])
                b_sb.append(bt)

            x_r = x_d[:].rearrange("(t c p) j -> t p c j", p=P, c=C)
            q_r = q_d[:].rearrange("(t s p) j -> t p s j", p=P, s=C)

            def stageA(t):
                """x load + transpose + MLP + g + distance matmuls -> sdT PSUM."""
                x_sb = acts.tile([P, C, 72], bf16, tag="x")
                nc.sync.dma_start(out=x_sb, in_=x_r[t])
                ptx = ptp.tile([72, B], bf16, tag="xtp")
                for c in range(C):
                    nc.tensor.transpose(
                        ptx[:, P * c : P * (c + 1)], x_sb[:, c, :], ident
                    )
                xT = acts.tile([72, B], bf16, tag="xT")
                nc.vector.tensor_copy(xT, ptx)

                h = [xT]
                ep = 0
                for l in range(7):
                    dout = DIMS[l + 1]
                    kc, mc = kc_l[l], mc_l[l]
                    relu = (l + 1) in RELU_LAYERS
                    hn = []
                    for m in range(mc):
                        pw = min(128, dout - 128 * m)
                        ps = pmm.tile([pw, B], f32, tag="mm")
                        for k in range(kc):
                            lhsT = w_sb[l][:, k * dout + 128 * m : k * dout + 128 * m + pw]
                            nc.tensor.matmul(
                                ps, lhsT, h[k], start=(k == 0), stop=(k == kc - 1)
                            )
                        ht = acts.tile([pw, B], bf16, tag=f"h{l + 1}m{m}")
                        bias_col = b_sb[l][:pw, m : m + 1]
                        if ep % 2 == 0:  # scalar engine (ACT)
                            nc.scalar.activation(
                                out=ht,
                                in_=ps,
                                func=AF.Relu if relu else AF.Identity,
                                bias=bias_col,
                                scale=1.0,
                            )
                        else:  # vector engine (DVE)
                            if relu:
                                nc.vector.tensor_scalar(
                                    out=ht,
                                    in0=ps,
                                    scalar1=bias_col,
                                    scalar2=0.0,
                                    op0=ALU.add,
                                    op1=ALU.max,
                                )
                            else:
                                nc.vector.tensor_scalar_add(ht, ps, bias_col)
                        ep += 1
                        hn.append(ht)
                    h = hn

                f0, f1 = h  # [128, B], [72, B] bf16
                g0 = acts.tile([128, B], bf16, tag="g0")
                nc.vector.tensor_mul(g0, f0, f0)
                g1 = acts.tile([72, B], bf16, tag="g1")
                nc.vector.tensor_mul(g1, f1, f1)

                sdT = psd.tile([72, B], f32, tag="sd")
                nc.tensor.matmul(sdT, ones[:128, :72], g0, start=True, stop=False)
                nc.tensor.matmul(sdT, ones[:72, :72], g1, start=False, stop=False)
                nc.tensor.matmul(sdT, cm2A, f0, start=False, stop=False)
                nc.tensor.matmul(sdT, cm2B, f1, start=False, stop=True)
                return sdT

            def stageB(t, sdT):
                """csq add + reciprocal + transpose back + normalize + store."""
                sd1 = acts.tile([72, B], f32, tag="sd1")
                nc.scalar.activation(
                    out=sd1, in_=sdT, func=AF.Identity, bias=csq1[:, 0:1], scale=1.0
                )
                nomT = acts.tile([72, B], f32, tag="nomT")
                nc.vector.reciprocal_approx_fast(out=nomT, in_=sd1)

                pq = ppq.tile([P, C, 72], f32, tag="pq")
                for s in range(C):
                    nc.tensor.transpose(
                        pq[:, s, :], nomT[:, P * s : P * (s + 1)], identf[:72, :72]
                    )
                rs4 = acts.tile([P, C], f32, tag="rs4")
                nc.vector.reduce_sum(rs4, pq, axis=AX.X)
                rr4 = acts.tile([P, C], f32, tag="rr4")
                nc.vector.reciprocal(rr4, rs4)
                rr_b = bass.AP(
                    tensor=rr4.tensor,
                    offset=rr4.offset,
                    ap=[rr4.ap[0], rr4.ap[1], [0, 72]],
                )
                qt = acts.tile([P, C, 72], f32, tag="qt")
                nc.vector.tensor_tensor(out=qt, in0=pq, in1=rr_b, op=ALU.mult)
                nc.sync.dma_start(out=q_r[t], in_=qt)

            prev = None
            for t in range(n_tiles):
                cur = (t, stageA(t))
                if prev is not None:
                    stageB(*prev)
                prev = cur
            stageB(*prev)

    nc.compile()
    return nc


def _prep_consts(ws, bs, center):
    """Host-side marshalling of the small replicated weights."""
    import ml_dtypes

    bf = ml_dtypes.bfloat16
    kc_l = [(DIMS[i] + 127) // 128 for i in range(7)]
    mc_l = [(DIMS[i + 1] + 127) // 128 for i in range(7)]
    consts = {}
    for l in range(7):
        din, dout = DIMS[l], DIMS[l + 1]
        w = np.ascontiguousarray(ws[l], dtype=np.float32)
        if din > 128:
            kc = kc_l[l]
            w = np.ascontiguousarray(
                w.reshape(kc, 128, dout).transpose(1, 0, 2).reshape(128, kc * dout)
            )
        consts[f"w{l + 1}"] = w.astype(bf)
        bt = np.zeros((128, mc_l[l]), dtype=np.float32)
        for m in range(mc_l[l]):
            pw = min(128, dout - 128 * m)
            bt[:pw, m] = bs[l][128 * m : 128 * m + pw]
        consts[f"b{l + 1}"] = bt
    c = np.asarray(center, dtype=np.float32)
    consts["cm2A"] = np.ascontiguousarray(-2.0 * c[:128, :]).astype(bf)
    consts["cm2B"] = np.ascontiguousarray(-2.0 * c[128:, :]).astype(bf)
    consts["csq1"] = np.ascontiguousarray(
        (1.0 + (c.astype(np.float64) ** 2).sum(axis=0)).reshape(72, 1)
    ).astype(np.float32)
    return consts


def kernel(
    inputs, w1, b1, w2, b2, w3, b3, w4, b4, w5, b5, w6, b6, w7, b7, center
):
    import ml_dtypes
    from concourse.bass_utils import run_bass_kernel_spmd

    x = np.asarray(inputs).astype(ml_dtypes.bfloat16)
    n = x.shape[0]
    n_loc = n // N_CORES
    key = n_loc
    if key not in _CACHE:
        _CACHE[key] = _build(n_loc)
    nc = _CACHE[key]

    consts = _prep_consts(
        [w1, w2, w3, w4, w5, w6, w7], [b1, b2, b3, b4, b5, b6, b7], center
    )
    in_maps = []
    for c in range(N_CORES):
        m = {"x": np.ascontiguousarray(x[c * n_loc : (c + 1) * n_loc])}
        m.update(consts)
        in_maps.append(m)
    res = run_bass_kernel_spmd(nc, in_maps, core_ids=list(range(N_CORES)))
    return np.concatenate([res.results[c]["q"] for c in range(N_CORES)], axis=0)
